# revision 1
# baseline (speedup 1.0000x reference)
"""Multi-head attention (B=2, S=2048, D=1024, H=16) on 8 Trainium2 NeuronCores.

Sharding: 2-way data parallel over batch x 4-way tensor parallel over heads.
Core c handles batch c//4 and heads [4*(c%4), 4*(c%4)+4).  Each core computes
its 4 heads' attention and a partial output projection; the host sums the 4
partials per batch element (the bias bo is only added by the g==0 cores).

Host-side prep passes activations and weights pre-transposed so every matmul
contracts over the SBUF partition dimension with contiguous DMA loads.
"""

from contextlib import ExitStack

import numpy as np

import concourse.mybir as mybir
import concourse.tile as tile
from concourse import bacc
from concourse import bass_utils
from concourse._compat import with_exitstack

F32 = mybir.dt.float32
F32R = mybir.dt.float32r
BF16 = mybir.dt.bfloat16

# "f32r": fp32 storage, float32r matmuls.  "bf16": bf16 storage + matmuls.
DT_MODE = "f32r"

D_MODEL = 1024
N_HEAD = 16
DK = 64
B = 2
S = 2048
N_CORES = 8
HPC = 4          # heads per core
DPC = HPC * DK   # 256 output dims per core
KC = D_MODEL // 128   # 8 contraction chunks of 128
SQ = 512         # sequence quarter
NSQ = S // SQ    # 4
NJB = S // 128   # 16 key blocks
NSB = S // 128   # 16 query/row blocks

if DT_MODE == "bf16":
    import ml_dtypes
    SB_DT = BF16
    IO_NP = ml_dtypes.bfloat16
else:
    # float32r = fp32 storage, single-pass (rounded) PE matmul at bf16 speed.
    # Producers must write float32r directly for the BIR verifier.
    SB_DT = F32R
    IO_NP = np.float32

# Attention-side matmul dtype.  bf16 here saves ~10us (pipelined weight
# loads) but raises absmax error from 4.8e-4 to 6.4e-3 (peaked-softmax
# queries don't average out score rounding) -- measured, not worth it.
ATT_DT = SB_DT


@with_exitstack
def build_mha(ctx: ExitStack, tc, ins, out_ap, loop_n=None):
    """Emit the per-core kernel.  loop_n wraps the whole compute body in a
    hardware For_i loop (used only for timing measurement)."""
    nc = tc.nc
    P = 128
    Exp = mybir.ActivationFunctionType.Exp
    Add = mybir.AluOpType.add

    xq = ins["xq_t"].rearrange("(kc p) s -> p kc s", p=P)
    xk = ins["xk_t"].rearrange("(kc p) s -> p kc s", p=P)
    xv = ins["xv_t"].rearrange("(kc p) s -> p kc s", p=P)
    out = out_ap.rearrange("(sb p) n -> p sb n", p=P)

    ec = ctx.enter_context
    cpool = ec(tc.tile_pool(name="consts", bufs=1))
    xpool = ec(tc.tile_pool(name="xs", bufs=10))
    qkpool = ec(tc.tile_pool(name="qk", bufs=1))
    vpool = ec(tc.tile_pool(name="vh", bufs=1))
    ptpool = ec(tc.tile_pool(name="pt", bufs=4))
    apool = ec(tc.tile_pool(name="attn", bufs=1))
    opool = ec(tc.tile_pool(name="outs", bufs=3))
    npool = ec(tc.tile_pool(name="nrm", bufs=4))
    accpool = ec(tc.tile_pool(name="acc", bufs=1))
    pp_ps = ec(tc.tile_pool(name="proj_ps", bufs=1, space="PSUM"))
    sc_ps = ec(tc.tile_pool(name="score_ps", bufs=2, space="PSUM"))
    at_ps = ec(tc.tile_pool(name="att_ps", bufs=2, space="PSUM"))

    # --- constants ---
    wq_sb = cpool.tile([P, KC, DPC], SB_DT, tag="wq")
    wk_sb = cpool.tile([P, KC, DPC], SB_DT, tag="wk")
    wv_sb = cpool.tile([P, KC, DPC], SB_DT, tag="wv")
    wo_sb = cpool.tile([P, 2, D_MODEL], SB_DT, tag="wo")
    # per-chunk weight loads on the scalar-engine DMA queue: the first K/V
    # projection matmuls only wait for their own chunk, and the x-tile
    # stream (sync queue) runs in parallel.  wo is only needed at the end.
    wq_ap = ins["wq_t"].rearrange("(kc p) m -> p kc m", p=P)
    wk_ap = ins["wk_t"].rearrange("(kc p) m -> p kc m", p=P)
    wv_ap = ins["wv_t"].rearrange("(kc p) m -> p kc m", p=P)
    for kc in range(KC):
        nc.scalar.dma_start(wk_sb[:, kc, :], wk_ap[:, kc, :])
        nc.scalar.dma_start(wv_sb[:, kc, :], wv_ap[:, kc, :])
    for kc in range(KC):
        nc.scalar.dma_start(wq_sb[:, kc, :], wq_ap[:, kc, :])
    nc.gpsimd.dma_start(wo_sb[:], ins["wo_t"].rearrange("(c p) n -> p c n", p=P))
    bq_sb = cpool.tile([P, 2], F32, tag="bq")
    bk_sb = cpool.tile([P, 2], F32, tag="bk")
    bv_sb = cpool.tile([P, DPC], F32, tag="bv")
    bo_sb = cpool.tile([P, D_MODEL], F32, tag="bo")
    nc.gpsimd.dma_start(bq_sb[:], ins["bq_p"][:])
    nc.gpsimd.dma_start(bk_sb[:], ins["bk_p"][:])
    nc.gpsimd.dma_start(bv_sb[:], ins["bv_b"][:])
    nc.gpsimd.dma_start(bo_sb[:], ins["bo_b"][:])

    # --- persistent activations ---
    qh_sb = qkpool.tile([P, 2, S], ATT_DT, tag="qh")   # [dk%128, head_pair, s]
    kh_sb = qkpool.tile([P, 2, S], ATT_DT, tag="kh")
    vh_sb = vpool.tile([P, NJB, HPC, DK + 1], ATT_DT, tag="vh")  # + ones col
    at_sb = apool.tile([P, 2, S], SB_DT, tag="at")    # attn out, transposed

    # walrus can't memset float32r; memset f32 then broadcast-copy
    ones1 = cpool.tile([P, 1], F32, tag="ones1")
    nc.vector.memset(ones1[:], 1.0)
    nc.vector.tensor_copy(
        vh_sb[:, :, :, DK : DK + 1],
        ones1[:, None, None, :].to_broadcast((P, NJB, HPC, 1)),
    )

    def _qk_quarter(x_ap, w_sb, b_sb, dst, sq, dma=None):
        dma = dma or nc.sync
        ps = pp_ps.tile([P, 1024], F32, tag="pp")
        for kc in range(KC):
            xt = xpool.tile([P, SQ], SB_DT, tag="xt")
            dma.dma_start(xt[:], x_ap[:, kc, sq * SQ : (sq + 1) * SQ])
            nc.tensor.matmul(
                ps[:, 0:512], w_sb[:, kc, 0:128], xt[:],
                start=(kc == 0), stop=(kc == KC - 1),
            )
            nc.tensor.matmul(
                ps[:, 512:1024], w_sb[:, kc, 128:256], xt[:],
                start=(kc == 0), stop=(kc == KC - 1),
            )
        nc.vector.tensor_scalar_add(
            dst[:, 0, sq * SQ : (sq + 1) * SQ], ps[:, 0:512], b_sb[:, 0:1]
        )
        nc.vector.tensor_scalar_add(
            dst[:, 1, sq * SQ : (sq + 1) * SQ], ps[:, 512:1024], b_sb[:, 1:2]
        )

    def _v_quarter(sq):
        # natural layout [s, dv]; row-blocks sharing a PSUM bank run their
        # accumulation groups sequentially over pre-loaded k-chunks
        ps = pp_ps.tile([P, 1024], F32, tag="pp")
        xts = []
        for kc in range(KC):
            xt = xpool.tile([P, SQ], SB_DT, tag="xt")
            nc.sync.dma_start(xt[:], xv[:, kc, sq * SQ : (sq + 1) * SQ])
            xts.append(xt)
        for sbi in range(4):
            for kc in range(KC):
                nc.tensor.matmul(
                    ps[:, sbi * 256 : (sbi + 1) * 256],
                    xts[kc][:, sbi * 128 : (sbi + 1) * 128],
                    wv_sb[:, kc, :],
                    start=(kc == 0), stop=(kc == KC - 1),
                )
            jb = sq * 4 + sbi
            nc.vector.tensor_tensor(
                vh_sb[:, jb, :, 0:DK],
                ps[:, sbi * 256 : (sbi + 1) * 256].rearrange("p (h d) -> p h d", h=HPC),
                bv_sb[:].rearrange("p (h d) -> p h d", h=HPC),
                Add,
            )

    # attention partial accumulators, one per (head, query-quarter);
    # row 64 carries the running sum(exp) for the softmax denominator
    acc_sb = [
        [accpool.tile([65, 512], F32, tag=f"acc{i5}_{h}", name=f"acc{i5}_{h}") for h in range(HPC)]
        for i5 in range(NSQ)
    ]

    def _attn_block(i5, t, jq):
        """4 key-blocks of attention for head pair t, query quarter i5."""
        i_sl = slice(i5 * SQ, (i5 + 1) * SQ)
        att_e = at_ps.tile([P, 512], F32, tag="att")
        att_o = at_ps.tile([P, 512], F32, tag="att")
        pts = []
        jbs = range(jq * 4, jq * 4 + 4)
        for n, jb in enumerate(jbs):
            sc = sc_ps.tile([P, 1024], F32, tag="sc")
            j_sl = slice(jb * 128, (jb + 1) * 128)
            nc.tensor.matmul(
                sc[:, 0:512], kh_sb[0:64, t, j_sl], qh_sb[0:64, t, i_sl],
                start=True, stop=True,
            )
            nc.tensor.matmul(
                sc[:, 512:1024], kh_sb[64:128, t, j_sl],
                qh_sb[64:128, t, i_sl], start=True, stop=True,
            )
            pt = ptpool.tile([P, 1024], ATT_DT, tag="pt")
            nc.scalar.activation(pt[:], sc[:], Exp, scale=1.0 / np.sqrt(DK))
            pts.append(pt)
            if n > 0:
                ptp = pts[n - 1]
                nc.tensor.matmul(
                    att_e[0:65, :], vh_sb[:, jb - 1, 2 * t, :],
                    ptp[:, 0:512], start=(n - 1 == 0), stop=False,
                )
                nc.tensor.matmul(
                    att_o[0:65, :], vh_sb[:, jb - 1, 2 * t + 1, :],
                    ptp[:, 512:1024], start=(n - 1 == 0), stop=False,
                )
        jb_last = jq * 4 + 3
        nc.tensor.matmul(
            att_e[0:65, :], vh_sb[:, jb_last, 2 * t, :],
            pts[-1][:, 0:512], start=False, stop=True,
        )
        nc.tensor.matmul(
            att_o[0:65, :], vh_sb[:, jb_last, 2 * t + 1, :],
            pts[-1][:, 512:1024], start=False, stop=True,
        )
        for h, aps in ((2 * t, att_e), (2 * t + 1, att_o)):
            acc = acc_sb[i5][h]
            if jq == 0:
                nc.vector.tensor_copy(acc[:], aps[0:65, :])
            else:
                nc.vector.tensor_tensor(acc[:], acc[:], aps[0:65, :], Add)

    def _normalize(i5):
        i_sl = slice(i5 * SQ, (i5 + 1) * SQ)
        for h in range(HPC):
            acc = acc_sb[i5][h]
            t = h // 2
            rc = npool.tile([1, 512], F32, tag="rc")
            nc.vector.reciprocal(rc[:], acc[64:65, :])
            bc = npool.tile([64, 512], F32, tag="bc")
            nc.gpsimd.partition_broadcast(bc[:], rc[:])
            if h % 2 == 0:
                nc.vector.tensor_mul(at_sb[0:64, t, i_sl], acc[0:64, :], bc[:])
            else:
                tm = npool.tile([64, 512], SB_DT, tag="tm")
                nc.vector.tensor_mul(tm[:], acc[0:64, :], bc[:])
                nc.sync.dma_start(at_sb[64:128, t, i_sl], tm[:])

    def _final(i5):
        for sbi in range(4):
            sb = i5 * 4 + sbi
            s_sl = slice(sb * 128, (sb + 1) * 128)
            po = pp_ps.tile([P, 1024], F32, tag="pp")
            for c in range(2):
                nc.tensor.matmul(
                    po[:, 0:512], at_sb[:, c, s_sl], wo_sb[:, c, 0:512],
                    start=(c == 0), stop=(c == 1),
                )
                nc.tensor.matmul(
                    po[:, 512:1024], at_sb[:, c, s_sl], wo_sb[:, c, 512:1024],
                    start=(c == 0), stop=(c == 1),
                )
            ot = opool.tile([P, 1024], F32, tag="ot")
            nc.vector.tensor_tensor(ot[:], po[:], bo_sb[:], Add)
            nc.sync.dma_start(out[:, sb, :], ot[:])

    def _compute():
        # Stream key/value quarters: as soon as K/V quarter jq is projected,
        # all heads' attention over those 4 key blocks runs and accumulates
        # (value-weighted sums + sum-exp) into SBUF accumulators.  Q quarters
        # are projected just-in-time during round 0; normalize + output
        # projection fold into the last round.
        # K/V quarter 0 first so the first attention round isn't stuck
        # behind the full Q DMA in the queue
        _qk_quarter(xk, wk_sb, bk_sb, kh_sb, 0)
        _v_quarter(0)
        for sq in range(NSQ):
            _qk_quarter(xq, wq_sb, bq_sb, qh_sb, sq)
        for jq in range(NSQ):
            if jq > 0:
                _qk_quarter(xk, wk_sb, bk_sb, kh_sb, jq)
                _v_quarter(jq)
            for i5 in range(NSQ):
                for t in range(2):
                    _attn_block(i5, t, jq)
                if jq == NSQ - 1:
                    _normalize(i5)
                    _final(i5)

    if loop_n is not None and loop_n > 1:
        with tc.For_i(0, loop_n, 1):
            _compute()
    else:
        _compute()


def shard_inputs(q, k, v, Wq, bq, Wk, bk, Wv, bv, Wo, bo):
    """Build the 8 per-core input maps from the full inputs."""
    def prep(a):
        return np.ascontiguousarray(np.asarray(a, np.float32)).astype(IO_NP)

    in_maps = []
    for c in range(N_CORES):
        b, g = divmod(c, 4)
        hs = slice(g * DPC, (g + 1) * DPC)
        bo_b = (
            np.broadcast_to(np.asarray(bo, np.float32), (128, D_MODEL))
            if g == 0
            else np.zeros((128, D_MODEL), np.float32)
        )
        in_maps.append({
            "xq_t": prep(np.asarray(q)[b].T),
            "xk_t": prep(np.asarray(k)[b].T),
            "xv_t": prep(np.asarray(v)[b].T),
            "wq_t": prep(np.asarray(Wq)[hs, :].T),
            "wk_t": prep(np.asarray(Wk)[hs, :].T),
            "wv_t": prep(np.asarray(Wv)[hs, :].T),
            "wo_t": prep(np.asarray(Wo)[:, hs].T),
            "bq_p": np.ascontiguousarray(
                np.asarray(bq, np.float32)[hs].reshape(2, 128).T),
            "bk_p": np.ascontiguousarray(
                np.asarray(bk, np.float32)[hs].reshape(2, 128).T),
            "bv_b": np.ascontiguousarray(
                np.broadcast_to(np.asarray(bv, np.float32)[hs], (128, DPC))),
            "bo_b": np.ascontiguousarray(bo_b),
        })
    return in_maps


_NC = None


def build_nc(loop_n=None):
    nc = bacc.Bacc(
        "TRN2",
        target_bir_lowering=False,
        debug=False,
        enable_asserts=False,
        num_devices=N_CORES,
    )
    ins = {}
    for name in ("xq_t", "xk_t", "xv_t"):
        ins[name] = nc.dram_tensor(
            name, [D_MODEL, S], SB_DT, kind="ExternalInput").ap()
    for name in ("wq_t", "wk_t", "wv_t"):
        ins[name] = nc.dram_tensor(
            name, [D_MODEL, DPC], SB_DT, kind="ExternalInput").ap()
    ins["wo_t"] = nc.dram_tensor(
        "wo_t", [DPC, D_MODEL], SB_DT, kind="ExternalInput").ap()
    ins["bq_p"] = nc.dram_tensor("bq_p", [128, 2], F32, kind="ExternalInput").ap()
    ins["bk_p"] = nc.dram_tensor("bk_p", [128, 2], F32, kind="ExternalInput").ap()
    ins["bv_b"] = nc.dram_tensor("bv_b", [128, DPC], F32, kind="ExternalInput").ap()
    ins["bo_b"] = nc.dram_tensor(
        "bo_b", [128, D_MODEL], F32, kind="ExternalInput").ap()
    out_ap = nc.dram_tensor("out", [S, D_MODEL], F32, kind="ExternalOutput").ap()
    with tile.TileContext(nc) as tc:
        build_mha(tc, ins, out_ap, loop_n=loop_n)
    nc.compile()
    return nc


def _get_nc():
    global _NC
    if _NC is None:
        _NC = build_nc()
    return _NC


def run_sharded(inputs, trace=False):
    nc = _get_nc()
    in_maps = shard_inputs(**inputs)
    res = bass_utils.run_bass_kernel_spmd(
        nc, in_maps, core_ids=list(range(N_CORES)), trace=trace
    )
    acc = np.zeros((B, S, D_MODEL), np.float64)
    for c in range(N_CORES):
        acc[c // 4] += res.results[c]["out"].astype(np.float64)
    return acc.astype(np.float32), res


def kernel(**inputs):
    out, _ = run_sharded(inputs, trace=False)
    return out



# revision 9
# speedup vs baseline: 1.1826x; 1.1826x over previous
"""Multi-head attention (B=2, S=2048, D=1024, H=16) on 8 Trainium2 NeuronCores.

Sharding: 2-way data parallel over batch x 4-way tensor parallel over heads.
Core c handles batch c//4 and heads [4*(c%4), 4*(c%4)+4).  Each core computes
its 4 heads' attention and a partial output projection; the host sums the 4
partials per batch element (the bias bo is only added by the g==0 cores).

Structure (v2): query-quarter-major attention with PSUM-resident accumulators.
For each 512-query quarter i5, all 16 key blocks accumulate the value-weighted
sums directly in PSUM (one 16-matmul accumulation group per head, the V ones
column carrying the softmax denominator in row 64), so the vector engine does
no per-block work.  The inner loop is scores (PE, the two dk=64 halves
row-packed) -> exp (ACT) -> att (PE).  The scalar engine's exp stream
(~1.15us per [128,1024] block) is the bottleneck; K/V/Q projections,
normalize and the output projection are interleaved into the jb loop so they
hide under it.

PSUM (8 banks): scores + projection chains share one 2-buf x [128,1024] pool
(4 banks); the 4 per-head attention accumulators take one bank each.
"""

from contextlib import ExitStack

import numpy as np

import concourse.mybir as mybir
import concourse.tile as tile
from concourse import bacc
from concourse import bass_utils
from concourse._compat import with_exitstack

F32 = mybir.dt.float32
F32R = mybir.dt.float32r
BF16 = mybir.dt.bfloat16

# "f32r": fp32 storage, float32r matmuls.  "bf16": bf16 storage + matmuls.
DT_MODE = "f32r"

D_MODEL = 1024
N_HEAD = 16
DK = 64
B = 2
S = 2048
N_CORES = 8
HPC = 4          # heads per core
DPC = HPC * DK   # 256 output dims per core
KC = D_MODEL // 128   # 8 contraction chunks of 128
SQ = 512         # sequence quarter
NSQ = S // SQ    # 4
NJB = S // 128   # 16 key blocks
NSB = S // 128   # 16 query/row blocks

if DT_MODE == "bf16":
    import ml_dtypes
    SB_DT = BF16
    IO_NP = ml_dtypes.bfloat16
else:
    # float32r = fp32 storage, single-pass (rounded) PE matmul at bf16 speed.
    SB_DT = F32R
    IO_NP = np.float32

ATT_DT = SB_DT


@with_exitstack
def build_mha(ctx: ExitStack, tc, ins, out_ap, loop_n=None):
    """Emit the per-core kernel.  loop_n wraps the whole compute body in a
    hardware For_i loop (used only for timing measurement)."""
    nc = tc.nc
    P = 128
    Exp = mybir.ActivationFunctionType.Exp
    Add = mybir.AluOpType.add

    xq = ins["xq_t"].rearrange("(kc p) s -> p kc s", p=P)
    xk = ins["xk_t"].rearrange("(kc p) s -> p kc s", p=P)
    xv = ins["xv_t"].rearrange("(kc p) s -> p kc s", p=P)
    out = out_ap.rearrange("(sb p) n -> p sb n", p=P)

    ec = ctx.enter_context
    cpool = ec(tc.tile_pool(name="consts", bufs=1))
    xpool = ec(tc.tile_pool(name="xs", bufs=10))
    qkpool = ec(tc.tile_pool(name="qk", bufs=1))
    vpool = ec(tc.tile_pool(name="vh", bufs=1))
    ptpool = ec(tc.tile_pool(name="pt", bufs=4))
    apool = ec(tc.tile_pool(name="attn", bufs=1))
    opool = ec(tc.tile_pool(name="outs", bufs=3))
    npool = ec(tc.tile_pool(name="nrm", bufs=4))
    # scores AND projection/output chains rotate through the same 2 slots
    sc_ps = ec(tc.tile_pool(name="score_ps", bufs=2, space="PSUM"))
    # 4 tags x bufs=1: one bank per head accumulator, slots reused across i5
    ac_ps = ec(tc.tile_pool(name="acc_ps", bufs=1, space="PSUM"))

    # --- constants ---
    wq_sb = cpool.tile([P, KC, DPC], SB_DT, tag="wq")
    wk_sb = cpool.tile([P, KC, DPC], SB_DT, tag="wk")
    wv_sb = cpool.tile([P, KC, DPC], SB_DT, tag="wv")
    wo_sb = cpool.tile([P, 2, D_MODEL], SB_DT, tag="wo")
    wq_ap = ins["wq_t"].rearrange("(kc p) m -> p kc m", p=P)
    wk_ap = ins["wk_t"].rearrange("(kc p) m -> p kc m", p=P)
    wv_ap = ins["wv_t"].rearrange("(kc p) m -> p kc m", p=P)
    for kc in range(KC):
        nc.scalar.dma_start(wk_sb[:, kc, :], wk_ap[:, kc, :])
        nc.scalar.dma_start(wv_sb[:, kc, :], wv_ap[:, kc, :])
    for kc in range(KC):
        nc.scalar.dma_start(wq_sb[:, kc, :], wq_ap[:, kc, :])
    nc.gpsimd.dma_start(wo_sb[:], ins["wo_t"].rearrange("(c p) n -> p c n", p=P))
    bq_sb = cpool.tile([P, 2], F32, tag="bq")
    bk_sb = cpool.tile([P, 2], F32, tag="bk")
    bv_sb = cpool.tile([P, DPC], F32, tag="bv")
    bo_sb = cpool.tile([P, D_MODEL], F32, tag="bo")
    nc.gpsimd.dma_start(bq_sb[:], ins["bq_p"][:])
    nc.gpsimd.dma_start(bk_sb[:], ins["bk_p"][:])
    nc.gpsimd.dma_start(bv_sb[:], ins["bv_b"][:])
    nc.gpsimd.dma_start(bo_sb[:], ins["bo_b"][:])

    # --- persistent activations ---
    qh_sb = qkpool.tile([P, 2, S], ATT_DT, tag="qh")   # [dk%128, head_pair, s]
    kh_sb = qkpool.tile([P, 2, S], ATT_DT, tag="kh")
    vh_sb = vpool.tile([P, NJB, HPC, DK + 1], ATT_DT, tag="vh")  # + ones col
    at_sb = apool.tile([P, 2, S], SB_DT, tag="at")    # attn out, transposed

    # walrus can't memset float32r; memset f32 then broadcast-copy
    ones1 = cpool.tile([P, 1], F32, tag="ones1")
    nc.vector.memset(ones1[:], 1.0)
    nc.vector.tensor_copy(
        vh_sb[:, :, :, DK : DK + 1],
        ones1[:, None, None, :].to_broadcast((P, NJB, HPC, 1)),
    )

    def _x_tiles(x_ap, sq, dma=None):
        dma = dma or nc.sync
        xts = []
        for kc in range(KC):
            xt = xpool.tile([P, SQ], SB_DT, tag="xt")
            dma.dma_start(xt[:], x_ap[:, kc, sq * SQ : (sq + 1) * SQ])
            xts.append(xt)
        return xts

    def _qk_quarter(x_ap, w_sb, b_sb, dst, sq):
        xts = _x_tiles(x_ap, sq)
        ps = sc_ps.tile([P, 1024], F32, tag="sc", name=f"pp{sq}")
        for kc in range(KC):
            nc.tensor.matmul(
                ps[:, 0:512], w_sb[:, kc, 0:128], xts[kc][:],
                start=(kc == 0), stop=(kc == KC - 1),
            )
            nc.tensor.matmul(
                ps[:, 512:1024], w_sb[:, kc, 128:256], xts[kc][:],
                start=(kc == 0), stop=(kc == KC - 1),
            )
        nc.vector.tensor_scalar_add(
            dst[:, 0, sq * SQ : (sq + 1) * SQ], ps[:, 0:512], b_sb[:, 0:1]
        )
        nc.vector.tensor_scalar_add(
            dst[:, 1, sq * SQ : (sq + 1) * SQ], ps[:, 512:1024], b_sb[:, 1:2]
        )

    def _v_quarter(sq):
        xts = _x_tiles(xv, sq)
        ps = sc_ps.tile([P, 1024], F32, tag="sc", name=f"vp{sq}")
        for sbi in range(4):
            for kc in range(KC):
                nc.tensor.matmul(
                    ps[:, sbi * 256 : (sbi + 1) * 256],
                    xts[kc][:, sbi * 128 : (sbi + 1) * 128],
                    wv_sb[:, kc, :],
                    start=(kc == 0), stop=(kc == KC - 1),
                )
            jb = sq * 4 + sbi
            nc.vector.tensor_tensor(
                vh_sb[:, jb, :, 0:DK],
                ps[:, sbi * 256 : (sbi + 1) * 256].rearrange(
                    "p (h d) -> p h d", h=HPC),
                bv_sb[:].rearrange("p (h d) -> p h d", h=HPC),
                Add,
            )

    def _attention_i5(i5, extra):
        """Full attention pass for query quarter i5; extra[jb] holds thunks
        (projection chains, normalize/final of the previous quarter) to
        interleave into the jb loop."""
        i_sl = slice(i5 * SQ, (i5 + 1) * SQ)
        acc = [ac_ps.tile([P, SQ], F32, tag=f"acc{h}", name=f"acc{i5}_{h}")
               for h in range(HPC)]

        def _att(jb):
            first = jb == 0
            last = jb == NJB - 1
            for h in range(HPC):
                t, o = divmod(h, 2)
                pt = pts[jb][t]
                nc.tensor.matmul(
                    acc[h][0:65, :], vh_sb[:, jb, h, :],
                    pt[:, 512 * o : 512 * (o + 1)],
                    start=first, stop=last,
                )

        pts = {}
        for jb in range(NJB):
            j_sl = slice(jb * 128, (jb + 1) * 128)
            pr = []
            for t in range(2):
                sc = sc_ps.tile([P, 1024], F32, tag="sc")
                # dk=64 halves row-packed: rows 0:64 / 64:128 run concurrently
                nc.tensor.matmul(
                    sc[:, 0:512], kh_sb[0:64, t, j_sl], qh_sb[0:64, t, i_sl],
                    start=True, stop=True,
                )
                nc.tensor.matmul(
                    sc[:, 512:1024], kh_sb[64:128, t, j_sl],
                    qh_sb[64:128, t, i_sl], start=True, stop=True,
                )
                pt = ptpool.tile([P, 1024], ATT_DT, tag="pt")
                nc.scalar.activation(pt[:], sc[:], Exp, scale=1.0 / np.sqrt(DK))
                pr.append(pt)
            pts[jb] = pr
            if jb > 0:
                _att(jb - 1)
                del pts[jb - 1]
            for fn in extra[jb]:
                fn()
        _att(NJB - 1)
        return acc

    def _normalize_head(i5, acc, h):
        i_sl = slice(i5 * SQ, (i5 + 1) * SQ)
        t, o = divmod(h, 2)
        rc = npool.tile([1, SQ], F32, tag="rc")
        nc.vector.reciprocal(rc[:], acc[h][64:65, :])
        bc = npool.tile([64, SQ], F32, tag="bc")
        nc.gpsimd.partition_broadcast(bc[:], rc[:])
        if o == 0:
            nc.vector.tensor_mul(at_sb[0:64, t, i_sl], acc[h][0:64, :], bc[:])
        else:
            tm = npool.tile([64, SQ], SB_DT, tag="tm")
            nc.vector.tensor_mul(tm[:], acc[h][0:64, :], bc[:])
            nc.sync.dma_start(at_sb[64:128, t, i_sl], tm[:])

    def _final_block(sb, ot):
        """Output projection for seq block sb into ot, then DMA out."""
        po = sc_ps.tile([P, 1024], F32, tag="sc", name=f"po{sb}")
        s_sl = slice(sb * 128, (sb + 1) * 128)
        for c in range(2):
            nc.tensor.matmul(
                po[:, 0:512], at_sb[:, c, s_sl], wo_sb[:, c, 0:512],
                start=(c == 0), stop=(c == 1),
            )
            nc.tensor.matmul(
                po[:, 512:1024], at_sb[:, c, s_sl], wo_sb[:, c, 512:1024],
                start=(c == 0), stop=(c == 1),
            )
        nc.vector.tensor_tensor(ot[:], po[:], bo_sb[:], Add)
        nc.sync.dma_start(out[:, sb, :], ot[:])

    def _emit_final(sb_extras, i5, acc):
        """Queue normalize + output projection of quarter i5 into extras.
        All normalize heads go at jb=0: they must be emitted before the next
        quarter's first write to the acc banks (WAR, bank-collision)."""
        for h in range(HPC):
            sb_extras[0].append(
                lambda h=h: _normalize_head(i5, acc, h))
        for sbi in range(4):
            sb = i5 * 4 + sbi

            def mk(sb=sb):
                ot = opool.tile([P, D_MODEL], F32, tag="ot", name=f"ot{sb}")
                _final_block(sb, ot)
            sb_extras[3 + 3 * sbi].append(mk)

    def _compute():
        # head: K/V/Q quarter 0 so attention can start immediately
        _qk_quarter(xk, wk_sb, bk_sb, kh_sb, 0)
        _v_quarter(0)
        _qk_quarter(xq, wq_sb, bq_sb, qh_sb, 0)

        carry = None
        for i5 in range(NSQ):
            extra = [[] for _ in range(NJB)]
            if i5 == 0:
                # stream the remaining K/V quarters into quarter 0's jb loop,
                # each completing just before the blocks that need it
                for qq in range(1, NSQ):
                    extra[4 * qq - 3].append(
                        lambda qq=qq: _qk_quarter(xk, wk_sb, bk_sb, kh_sb, qq))
                    extra[4 * qq - 2].append(lambda qq=qq: _v_quarter(qq))
            else:
                _emit_final(extra, i5 - 1, carry)
            if i5 < NSQ - 1:
                extra[13].append(
                    lambda i5=i5: _qk_quarter(xq, wq_sb, bq_sb, qh_sb, i5 + 1))
            carry = _attention_i5(i5, extra)

        # tail: normalize + output projection of the last quarter
        for h in range(HPC):
            _normalize_head(NSQ - 1, carry, h)
        for sbi in range(4):
            sb = (NSQ - 1) * 4 + sbi
            ot = opool.tile([P, D_MODEL], F32, tag="ot", name=f"otl{sb}")
            _final_block(sb, ot)

    if loop_n is not None and loop_n > 1:
        with tc.For_i(0, loop_n, 1):
            _compute()
    else:
        _compute()


def shard_inputs(q, k, v, Wq, bq, Wk, bk, Wv, bv, Wo, bo):
    """Build the 8 per-core input maps from the full inputs."""
    def prep(a):
        return np.ascontiguousarray(np.asarray(a, np.float32)).astype(IO_NP)

    in_maps = []
    for c in range(N_CORES):
        b, g = divmod(c, 4)
        hs = slice(g * DPC, (g + 1) * DPC)
        bo_b = (
            np.broadcast_to(np.asarray(bo, np.float32), (128, D_MODEL))
            if g == 0
            else np.zeros((128, D_MODEL), np.float32)
        )
        in_maps.append({
            "xq_t": prep(np.asarray(q)[b].T),
            "xk_t": prep(np.asarray(k)[b].T),
            "xv_t": prep(np.asarray(v)[b].T),
            "wq_t": prep(np.asarray(Wq)[hs, :].T),
            "wk_t": prep(np.asarray(Wk)[hs, :].T),
            "wv_t": prep(np.asarray(Wv)[hs, :].T),
            "wo_t": prep(np.asarray(Wo)[:, hs].T),
            "bq_p": np.ascontiguousarray(
                np.asarray(bq, np.float32)[hs].reshape(2, 128).T),
            "bk_p": np.ascontiguousarray(
                np.asarray(bk, np.float32)[hs].reshape(2, 128).T),
            "bv_b": np.ascontiguousarray(
                np.broadcast_to(np.asarray(bv, np.float32)[hs], (128, DPC))),
            "bo_b": np.ascontiguousarray(bo_b),
        })
    return in_maps


_NC = None


def build_nc(loop_n=None):
    nc = bacc.Bacc(
        "TRN2",
        target_bir_lowering=False,
        debug=False,
        enable_asserts=False,
        num_devices=N_CORES,
    )
    ins = {}
    for name in ("xq_t", "xk_t", "xv_t"):
        ins[name] = nc.dram_tensor(
            name, [D_MODEL, S], SB_DT, kind="ExternalInput").ap()
    for name in ("wq_t", "wk_t", "wv_t"):
        ins[name] = nc.dram_tensor(
            name, [D_MODEL, DPC], SB_DT, kind="ExternalInput").ap()
    ins["wo_t"] = nc.dram_tensor(
        "wo_t", [DPC, D_MODEL], SB_DT, kind="ExternalInput").ap()
    ins["bq_p"] = nc.dram_tensor("bq_p", [128, 2], F32, kind="ExternalInput").ap()
    ins["bk_p"] = nc.dram_tensor("bk_p", [128, 2], F32, kind="ExternalInput").ap()
    ins["bv_b"] = nc.dram_tensor("bv_b", [128, DPC], F32, kind="ExternalInput").ap()
    ins["bo_b"] = nc.dram_tensor(
        "bo_b", [128, D_MODEL], F32, kind="ExternalInput").ap()
    out_ap = nc.dram_tensor("out", [S, D_MODEL], F32, kind="ExternalOutput").ap()
    with tile.TileContext(nc) as tc:
        build_mha(tc, ins, out_ap, loop_n=loop_n)
    nc.compile()
    return nc


def _get_nc():
    global _NC
    if _NC is None:
        _NC = build_nc()
    return _NC


def run_sharded(inputs, trace=False):
    nc = _get_nc()
    in_maps = shard_inputs(**inputs)
    res = bass_utils.run_bass_kernel_spmd(
        nc, in_maps, core_ids=list(range(N_CORES)), trace=trace
    )
    acc = np.zeros((B, S, D_MODEL), np.float64)
    for c in range(N_CORES):
        acc[c // 4] += res.results[c]["out"].astype(np.float64)
    return acc.astype(np.float32), res


def kernel(**inputs):
    out, _ = run_sharded(inputs, trace=False)
    return out


# revision 14
# speedup vs baseline: 1.3211x; 1.1171x over previous
"""Multi-head attention (B=2, S=2048, D=1024, H=16) on 8 Trainium2 NeuronCores.

Sharding: 2-way data parallel over batch x 4-way tensor parallel over heads.
Core c handles batch c//4 and heads [4*(c%4), 4*(c%4)+4).  Each core computes
its 4 heads' attention and a partial output projection; the host sums the 4
partials per batch element (the bias bo is only added by the g==0 cores).

Structure (v2): query-quarter-major attention with PSUM-resident accumulators.
For each 512-query quarter i5, all 16 key blocks accumulate the value-weighted
sums directly in PSUM (one 16-matmul accumulation group per head, the V ones
column carrying the softmax denominator in row 64), so the vector engine does
no per-block work.  The inner loop is scores (PE, the two dk=64 halves
row-packed) -> exp (ACT) -> att (PE).  The scalar engine's exp stream
(~1.15us per [128,1024] block) is the bottleneck; K/V/Q projections,
normalize and the output projection are interleaved into the jb loop so they
hide under it.

PSUM (8 banks): scores + projection chains share one 2-buf x [128,1024] pool
(4 banks); the 4 per-head attention accumulators take one bank each.
"""

from contextlib import ExitStack

import numpy as np

import concourse.mybir as mybir
import concourse.tile as tile
from concourse import bacc
from concourse import bass_utils
from concourse._compat import with_exitstack

F32 = mybir.dt.float32
F32R = mybir.dt.float32r
BF16 = mybir.dt.bfloat16

# "f32r": fp32 storage, float32r matmuls.  "bf16": bf16 storage + matmuls.
DT_MODE = "bf16"

D_MODEL = 1024
N_HEAD = 16
DK = 64
B = 2
S = 2048
N_CORES = 8
HPC = 4          # heads per core
DPC = HPC * DK   # 256 output dims per core
KC = D_MODEL // 128   # 8 contraction chunks of 128
SQ = 512         # sequence quarter
NSQ = S // SQ    # 4
NJB = S // 128   # 16 key blocks
NSB = S // 128   # 16 query/row blocks

if DT_MODE == "bf16":
    import ml_dtypes
    SB_DT = BF16
    IO_NP = ml_dtypes.bfloat16
else:
    # float32r = fp32 storage, single-pass (rounded) PE matmul at bf16 speed.
    SB_DT = F32R
    IO_NP = np.float32

ATT_DT = SB_DT


@with_exitstack
def build_mha(ctx: ExitStack, tc, ins, out_ap, loop_n=None):
    """Emit the per-core kernel.  loop_n wraps the whole compute body in a
    hardware For_i loop (used only for timing measurement)."""
    nc = tc.nc
    P = 128
    Exp = mybir.ActivationFunctionType.Exp
    Add = mybir.AluOpType.add

    xq = ins["xq_t"].rearrange("(kc p) s -> p kc s", p=P)
    xk = ins["xk_t"].rearrange("(kc p) s -> p kc s", p=P)
    xv = ins["xv_t"].rearrange("(kc p) s -> p kc s", p=P)
    out = out_ap.rearrange("(sb p) n -> p sb n", p=P)

    ec = ctx.enter_context
    cpool = ec(tc.tile_pool(name="consts", bufs=1))
    xpool = ec(tc.tile_pool(name="xs", bufs=10))
    qkpool = ec(tc.tile_pool(name="qk", bufs=1))
    vpool = ec(tc.tile_pool(name="vh", bufs=1))
    ptpool = ec(tc.tile_pool(name="pt", bufs=4))
    apool = ec(tc.tile_pool(name="attn", bufs=1))
    opool = ec(tc.tile_pool(name="outs", bufs=3))
    npool = ec(tc.tile_pool(name="nrm", bufs=4))
    # scores AND projection/output chains rotate through the same 2 slots
    sc_ps = ec(tc.tile_pool(name="score_ps", bufs=2, space="PSUM"))
    # 4 tags x bufs=1: one bank per head accumulator, slots reused across i5
    ac_ps = ec(tc.tile_pool(name="acc_ps", bufs=1, space="PSUM"))

    # --- constants ---
    wq_sb = cpool.tile([P, KC, DPC], SB_DT, tag="wq")
    wk_sb = cpool.tile([P, KC, DPC], SB_DT, tag="wk")
    wv_sb = cpool.tile([P, KC, DPC], SB_DT, tag="wv")
    wo_sb = cpool.tile([P, 2, D_MODEL], SB_DT, tag="wo")
    wq_ap = ins["wq_t"].rearrange("(kc p) m -> p kc m", p=P)
    wk_ap = ins["wk_t"].rearrange("(kc p) m -> p kc m", p=P)
    wv_ap = ins["wv_t"].rearrange("(kc p) m -> p kc m", p=P)
    for kc in range(KC):
        nc.scalar.dma_start(wk_sb[:, kc, :], wk_ap[:, kc, :])
        nc.scalar.dma_start(wq_sb[:, kc, :], wq_ap[:, kc, :])
    for kc in range(KC):
        nc.scalar.dma_start(wv_sb[:, kc, :], wv_ap[:, kc, :])
    nc.gpsimd.dma_start(wo_sb[:], ins["wo_t"].rearrange("(c p) n -> p c n", p=P))
    bq_sb = cpool.tile([P, 2], F32, tag="bq")
    bk_sb = cpool.tile([P, 2], F32, tag="bk")
    bv_sb = cpool.tile([P, DPC], F32, tag="bv")
    bo_sb = cpool.tile([P, D_MODEL], F32, tag="bo")
    nc.gpsimd.dma_start(bq_sb[:], ins["bq_p"][:])
    nc.gpsimd.dma_start(bk_sb[:], ins["bk_p"][:])
    nc.gpsimd.dma_start(bv_sb[:], ins["bv_b"][:])
    nc.gpsimd.dma_start(bo_sb[:], ins["bo_b"][:])

    # --- persistent activations ---
    qh_sb = qkpool.tile([P, 2, S], ATT_DT, tag="qh")   # [dk%128, head_pair, s]
    kh_sb = qkpool.tile([P, 2, S], ATT_DT, tag="kh")
    vh_sb = vpool.tile([P, NJB, HPC, DK + 1], ATT_DT, tag="vh")  # + ones col
    at_sb = apool.tile([P, 2, S], SB_DT, tag="at")    # attn out, transposed

    # walrus can't memset float32r; memset f32 then broadcast-copy
    ones1 = cpool.tile([P, 1], F32, tag="ones1")
    nc.vector.memset(ones1[:], 1.0)
    nc.vector.tensor_copy(
        vh_sb[:, :, :, DK : DK + 1],
        ones1[:, None, None, :].to_broadcast((P, NJB, HPC, 1)),
    )

    def _x_tiles(x_ap, sq, queues=None):
        # split the 8 chunk loads across DMA queues to halve the feed latency
        queues = queues or (nc.sync, nc.gpsimd)
        xts = []
        for kc in range(KC):
            xt = xpool.tile([P, SQ], SB_DT, tag="xt")
            queues[kc % len(queues)].dma_start(
                xt[:], x_ap[:, kc, sq * SQ : (sq + 1) * SQ])
            xts.append(xt)
        return xts

    def _qk_quarter(x_ap, w_sb, b_sb, dst, sq, queues=None):
        xts = _x_tiles(x_ap, sq, queues)
        ps = sc_ps.tile([P, 1024], F32, tag="sc", name=f"pp{sq}")
        for kc in range(KC):
            nc.tensor.matmul(
                ps[:, 0:512], w_sb[:, kc, 0:128], xts[kc][:],
                start=(kc == 0), stop=(kc == KC - 1),
            )
            nc.tensor.matmul(
                ps[:, 512:1024], w_sb[:, kc, 128:256], xts[kc][:],
                start=(kc == 0), stop=(kc == KC - 1),
            )
        nc.vector.tensor_scalar_add(
            dst[:, 0, sq * SQ : (sq + 1) * SQ], ps[:, 0:512], b_sb[:, 0:1]
        )
        nc.vector.tensor_scalar_add(
            dst[:, 1, sq * SQ : (sq + 1) * SQ], ps[:, 512:1024], b_sb[:, 1:2]
        )

    def _v_quarter(sq, queues=None):
        xts = _x_tiles(xv, sq, queues)
        ps = sc_ps.tile([P, 1024], F32, tag="sc", name=f"vp{sq}")
        for sbi in range(4):
            for kc in range(KC):
                nc.tensor.matmul(
                    ps[:, sbi * 256 : (sbi + 1) * 256],
                    xts[kc][:, sbi * 128 : (sbi + 1) * 128],
                    wv_sb[:, kc, :],
                    start=(kc == 0), stop=(kc == KC - 1),
                )
            jb = sq * 4 + sbi
            nc.vector.tensor_tensor(
                vh_sb[:, jb, :, 0:DK],
                ps[:, sbi * 256 : (sbi + 1) * 256].rearrange(
                    "p (h d) -> p h d", h=HPC),
                bv_sb[:].rearrange("p (h d) -> p h d", h=HPC),
                Add,
            )

    def _attention_i5(i5, extra):
        """Full attention pass for query quarter i5; extra[jb] holds thunks
        (projection chains, normalize/final of the previous quarter) to
        interleave into the jb loop."""
        i_sl = slice(i5 * SQ, (i5 + 1) * SQ)
        acc = [ac_ps.tile([P, SQ], F32, tag=f"acc{h}", name=f"acc{i5}_{h}")
               for h in range(HPC)]

        def _att(jb):
            first = jb == 0
            last = jb == NJB - 1
            for h in range(HPC):
                t, o = divmod(h, 2)
                pt = pts[jb][t]
                nc.tensor.matmul(
                    acc[h][0:65, :], vh_sb[:, jb, h, :],
                    pt[:, 512 * o : 512 * (o + 1)],
                    start=first, stop=last,
                )

        pts = {}
        for jb in range(NJB):
            j_sl = slice(jb * 128, (jb + 1) * 128)
            pr = []
            for t in range(2):
                sc = sc_ps.tile([P, 1024], F32, tag="sc")
                # dk=64 halves row-packed: rows 0:64 / 64:128 run concurrently
                nc.tensor.matmul(
                    sc[:, 0:512], kh_sb[0:64, t, j_sl], qh_sb[0:64, t, i_sl],
                    start=True, stop=True,
                )
                nc.tensor.matmul(
                    sc[:, 512:1024], kh_sb[64:128, t, j_sl],
                    qh_sb[64:128, t, i_sl], start=True, stop=True,
                )
                pt = ptpool.tile([P, 1024], ATT_DT, tag="pt")
                nc.scalar.activation(pt[:], sc[:], Exp, scale=1.0 / np.sqrt(DK))
                pr.append(pt)
            pts[jb] = pr
            if jb > 0:
                _att(jb - 1)
                del pts[jb - 1]
            for fn in extra[jb]:
                fn()
        _att(NJB - 1)
        return acc

    def _normalize_head(i5, acc, h):
        i_sl = slice(i5 * SQ, (i5 + 1) * SQ)
        t, o = divmod(h, 2)
        rc = npool.tile([1, SQ], F32, tag="rc")
        nc.vector.reciprocal(rc[:], acc[h][64:65, :])
        bc = npool.tile([64, SQ], F32, tag="bc")
        nc.gpsimd.partition_broadcast(bc[:], rc[:])
        if o == 0:
            nc.vector.tensor_mul(at_sb[0:64, t, i_sl], acc[h][0:64, :], bc[:])
        else:
            tm = npool.tile([64, SQ], SB_DT, tag="tm")
            nc.vector.tensor_mul(tm[:], acc[h][0:64, :], bc[:])
            nc.sync.dma_start(at_sb[64:128, t, i_sl], tm[:])

    def _final_block(sb, ot):
        """Output projection for seq block sb into ot, then DMA out."""
        po = sc_ps.tile([P, 1024], F32, tag="sc", name=f"po{sb}")
        s_sl = slice(sb * 128, (sb + 1) * 128)
        for c in range(2):
            nc.tensor.matmul(
                po[:, 0:512], at_sb[:, c, s_sl], wo_sb[:, c, 0:512],
                start=(c == 0), stop=(c == 1),
            )
            nc.tensor.matmul(
                po[:, 512:1024], at_sb[:, c, s_sl], wo_sb[:, c, 512:1024],
                start=(c == 0), stop=(c == 1),
            )
        nc.vector.tensor_tensor(ot[:], po[:], bo_sb[:], Add)
        nc.sync.dma_start(out[:, sb, :], ot[:])

    def _emit_final(sb_extras, i5, acc):
        """Queue normalize + output projection of quarter i5 into extras.
        All normalize heads go at jb=0: they must be emitted before the next
        quarter's first write to the acc banks (WAR, bank-collision)."""
        for h in range(HPC):
            sb_extras[0].append(
                lambda h=h: _normalize_head(i5, acc, h))
        for sbi in range(4):
            sb = i5 * 4 + sbi

            def mk(sb=sb):
                ot = opool.tile([P, D_MODEL], F32, tag="ot", name=f"ot{sb}")
                _final_block(sb, ot)
            sb_extras[3 + 3 * sbi].append(mk)

    def _compute():
        # head: K/V/Q quarter 0 so attention can start immediately.
        # K0/Q0 load on disjoint queues so the first scores fire ASAP.
        _qk_quarter(xk, wk_sb, bk_sb, kh_sb, 0, queues=(nc.sync,))
        _qk_quarter(xq, wq_sb, bq_sb, qh_sb, 0, queues=(nc.gpsimd,))
        _v_quarter(0)

        carry = None
        for i5 in range(NSQ):
            extra = [[] for _ in range(NJB)]
            if i5 == 0:
                # stream the remaining K/V quarters into quarter 0's jb loop,
                # each completing just before the blocks that need it
                for qq in range(1, NSQ):
                    extra[4 * qq - 3].append(
                        lambda qq=qq: _qk_quarter(xk, wk_sb, bk_sb, kh_sb, qq))
                    extra[4 * qq - 2].append(lambda qq=qq: _v_quarter(qq))
            else:
                _emit_final(extra, i5 - 1, carry)
            if i5 < NSQ - 1:
                extra[13].append(
                    lambda i5=i5: _qk_quarter(xq, wq_sb, bq_sb, qh_sb, i5 + 1))
            carry = _attention_i5(i5, extra)

        # tail: normalize + output projection of the last quarter
        for h in range(HPC):
            _normalize_head(NSQ - 1, carry, h)
        for sbi in range(4):
            sb = (NSQ - 1) * 4 + sbi
            ot = opool.tile([P, D_MODEL], F32, tag="ot", name=f"otl{sb}")
            _final_block(sb, ot)

    if loop_n is not None and loop_n > 1:
        with tc.For_i(0, loop_n, 1):
            _compute()
    else:
        _compute()


def shard_inputs(q, k, v, Wq, bq, Wk, bk, Wv, bv, Wo, bo):
    """Build the 8 per-core input maps from the full inputs."""
    def prep(a):
        return np.ascontiguousarray(np.asarray(a, np.float32)).astype(IO_NP)

    in_maps = []
    for c in range(N_CORES):
        b, g = divmod(c, 4)
        hs = slice(g * DPC, (g + 1) * DPC)
        bo_b = (
            np.broadcast_to(np.asarray(bo, np.float32), (128, D_MODEL))
            if g == 0
            else np.zeros((128, D_MODEL), np.float32)
        )
        in_maps.append({
            "xq_t": prep(np.asarray(q)[b].T),
            "xk_t": prep(np.asarray(k)[b].T),
            "xv_t": prep(np.asarray(v)[b].T),
            "wq_t": prep(np.asarray(Wq)[hs, :].T),
            "wk_t": prep(np.asarray(Wk)[hs, :].T),
            "wv_t": prep(np.asarray(Wv)[hs, :].T),
            "wo_t": prep(np.asarray(Wo)[:, hs].T),
            "bq_p": np.ascontiguousarray(
                np.asarray(bq, np.float32)[hs].reshape(2, 128).T),
            "bk_p": np.ascontiguousarray(
                np.asarray(bk, np.float32)[hs].reshape(2, 128).T),
            "bv_b": np.ascontiguousarray(
                np.broadcast_to(np.asarray(bv, np.float32)[hs], (128, DPC))),
            "bo_b": np.ascontiguousarray(bo_b),
        })
    return in_maps


_NC = None


def build_nc(loop_n=None):
    nc = bacc.Bacc(
        "TRN2",
        target_bir_lowering=False,
        debug=False,
        enable_asserts=False,
        num_devices=N_CORES,
    )
    ins = {}
    for name in ("xq_t", "xk_t", "xv_t"):
        ins[name] = nc.dram_tensor(
            name, [D_MODEL, S], SB_DT, kind="ExternalInput").ap()
    for name in ("wq_t", "wk_t", "wv_t"):
        ins[name] = nc.dram_tensor(
            name, [D_MODEL, DPC], SB_DT, kind="ExternalInput").ap()
    ins["wo_t"] = nc.dram_tensor(
        "wo_t", [DPC, D_MODEL], SB_DT, kind="ExternalInput").ap()
    ins["bq_p"] = nc.dram_tensor("bq_p", [128, 2], F32, kind="ExternalInput").ap()
    ins["bk_p"] = nc.dram_tensor("bk_p", [128, 2], F32, kind="ExternalInput").ap()
    ins["bv_b"] = nc.dram_tensor("bv_b", [128, DPC], F32, kind="ExternalInput").ap()
    ins["bo_b"] = nc.dram_tensor(
        "bo_b", [128, D_MODEL], F32, kind="ExternalInput").ap()
    out_ap = nc.dram_tensor("out", [S, D_MODEL], F32, kind="ExternalOutput").ap()
    with tile.TileContext(nc) as tc:
        build_mha(tc, ins, out_ap, loop_n=loop_n)
    nc.compile()
    return nc


def _get_nc():
    global _NC
    if _NC is None:
        _NC = build_nc()
    return _NC


def run_sharded(inputs, trace=False):
    nc = _get_nc()
    in_maps = shard_inputs(**inputs)
    res = bass_utils.run_bass_kernel_spmd(
        nc, in_maps, core_ids=list(range(N_CORES)), trace=trace
    )
    acc = np.zeros((B, S, D_MODEL), np.float64)
    for c in range(N_CORES):
        acc[c // 4] += res.results[c]["out"].astype(np.float64)
    return acc.astype(np.float32), res


def kernel(**inputs):
    out, _ = run_sharded(inputs, trace=False)
    return out
